# revision 29
# baseline (speedup 1.0000x reference)
"""FCOS detection decode kernel for Trainium2 (8 NeuronCores, data parallel).

Full inputs:
  bbox       [16, 128, 128, 4]  f32
  center     [16, 128, 128, 1]  f32
  cls_logits [16, 128, 128, 80] f32
Full outputs (tuple, matching the reference):
  xywh [16, 16384, 4] f32
  idx  [16, 16384]    int32
  conf [16, 16384]    f32

Sharding: batch dim across 8 cores (2 batches per core), no communication.

Per-core layout: partition dim = H (128 rows), free dim = W*channels.
  - exp/sigmoid/sqrt on ScalarE (ACT)
  - ltrb->xywh linear combinations on VectorE with a precomputed (cx, cy) tile
  - class max via tensor_reduce(max) over C, W-chunked (fine chunks first so
    compute starts as soon as the first cls DMA slice lands)
  - argmax via (cls >= max) mask (bf16) * descending iota (bf16, 2x mode),
    then a bf16 pairwise-max tree (tensor_tensor max is 2x on bf16 while
    tensor_reduce is always 1x) and idx = 80 - enc on ACT. Descending iota
    makes ties resolve to the first (lowest) class index, matching jnp.argmax.
"""

import math

import numpy as np

NB_FULL = 16
N_CORES = 8
NB = NB_FULL // N_CORES  # batches per core
H = 128
W = 128
C = 80

_CACHE = {}



def _register_enc_op():
    """Custom DVE op: enc = select(cls >= m, -within_page_pos, -FLT_MAX).

    One 1x pass replaces the is_ge mask pass + the iota multiply. SubIdx is
    the page (cell) counter, Idx the global stream position, so
    C0*SubIdx - Idx = -(position within the 80-class page). Values are
    -79..0 (bf16-exact); reduce_max(enc) = -argmax with first-occurrence
    ties, matching jnp.argmax.
    """
    import numpy as np
    from concourse import dve_ops
    from concourse.dve_spec import (
        Spec, Src0, Src1, C0, MaxNeg, select, lower, Idx, SubIdx,
    )
    from concourse.dve_uop import DveOpSpec

    for o in dve_ops.OPS:
        if o.name == "ENC_ARGMAX_ANT":
            return o

    def ref(in0, in1, s0, s1, imm2):
        P, S, N = in0.shape
        pos = np.arange(S * N).reshape(1, S, N)
        sub = np.repeat(np.arange(S), N).reshape(1, S, N)
        s0v = float(s0) if not isinstance(s0, np.ndarray) else float(s0.flat[0])
        return np.where(in0 >= in1, s0v * sub - pos, -3.4028235e38)

    spec = Spec(
        body=select(Src0 >= Src1, (C0 * SubIdx) - Idx, MaxNeg), reference=ref
    )
    shas = {}
    for ver in ("v3", "v4"):
        tmp = DveOpSpec(
            name="ENC_ARGMAX_ANT", opcode=0, uops=lower(spec, ver=ver), rd1_en=True
        )
        shas[ver] = tmp.sha(ver)
    op = dve_ops.DveOp("ENC_ARGMAX_ANT", spec, subdim=True, uops_sha=shas)
    dve_ops.OPS.append(op)
    dve_ops._SUB_OPCODE_FOR_NAME[op.name] = (
        dve_ops._CUSTOM_DVE_ROW_BASE + len(dve_ops.OPS) - 1
    )
    dve_ops.CUSTOM_DVE_SPECS[op.name] = op.spec
    return op


def _build(repeat=1, chunk_plan=None, loop_n=0, cls_bufs=4, mask_bufs=1, enc_bufs=2, fused_enc=True):

    """Build the per-core Bass/Tile program.

    chunk_plan: per-batch list of W-chunk widths for the cls pipeline.
      Fine first chunks let compute start early; coarse later chunks cut
      per-instruction overhead.
    repeat / loop_n: benchmarking only (python-unrolled / hardware For_i).
    """
    from contextlib import ExitStack, nullcontext

    import concourse.bacc as bacc
    import concourse.mybir as mybir
    import concourse.tile as tile

    f32 = mybir.dt.float32
    bf16 = mybir.dt.bfloat16
    i32 = mybir.dt.int32
    Alu = mybir.AluOpType
    Act = mybir.ActivationFunctionType

    if chunk_plan is None:
        chunk_plan = [[8, 8, 16, 32, 32, 32], [64, 64]]
    assert len(chunk_plan) == NB and all(sum(p) == W for p in chunk_plan)

    enc_op = _register_enc_op() if fused_enc else None

    nc = bacc.Bacc("TRN2", target_bir_lowering=False, debug=False)

    bbox_d = nc.dram_tensor("bbox", [NB, H, W, 4], f32, kind="ExternalInput")
    ctr_d = nc.dram_tensor("center", [NB, H, W, 1], f32, kind="ExternalInput")
    cls_d = nc.dram_tensor("cls", [NB, H, W, C], f32, kind="ExternalInput")
    xywh_d = nc.dram_tensor("xywh", [NB, H * W, 4], f32, kind="ExternalOutput")
    idx_d = nc.dram_tensor("idx", [NB, H * W], i32, kind="ExternalOutput")
    conf_d = nc.dram_tensor("conf", [NB, H * W], f32, kind="ExternalOutput")

    # (batch, w_start, w_width) in pipeline order
    chunks = []
    for b, plan in enumerate(chunk_plan):
        w0 = 0
        for cw in plan:
            chunks.append((b, w0, cw))
            w0 += cw

    with tile.TileContext(nc) as tc, ExitStack() as ctx:
        const_pool = ctx.enter_context(tc.tile_pool(name="const", bufs=1))
        cls_pool = ctx.enter_context(tc.tile_pool(name="cls", bufs=cls_bufs))
        mask_pool = ctx.enter_context(tc.tile_pool(name="maskp", bufs=mask_bufs))
        enc_pool = ctx.enter_context(tc.tile_pool(name="encp", bufs=enc_bufs))
        io_pool = ctx.enter_context(tc.tile_pool(name="io", bufs=2))
        scr_pool = ctx.enter_context(tc.tile_pool(name="scr", bufs=1))

        # Descending iota 80..1 (value 80-c), as bf16 (exact for ints <= 256).
        iota_i = const_pool.tile([128, C], i32)
        nc.gpsimd.iota(iota_i[:], pattern=[[-1, C]], base=C, channel_multiplier=0)
        iota_b = const_pool.tile([128, C], bf16)
        nc.vector.tensor_copy(iota_b[:], iota_i[:])

        # cxy tile [128, W, 2]: (cx=8w+4 along free, cy=8h+4 along partitions)
        cxy_i = const_pool.tile([128, W * 2], i32)
        cxy_i3 = cxy_i[:].rearrange("p (w k) -> p w k", k=2)
        nc.gpsimd.iota(cxy_i3[:, :, 0], pattern=[[1, W]], base=0, channel_multiplier=0)
        nc.gpsimd.iota(cxy_i3[:, :, 1], pattern=[[0, W]], base=0, channel_multiplier=1)
        cxy_f = const_pool.tile([128, W * 2], f32)
        nc.vector.tensor_scalar(cxy_f[:], cxy_i[:], 8.0, 4.0, op0=Alu.mult, op1=Alu.add)
        cxy_f3 = cxy_f[:].rearrange("p (w k) -> p w k", k=2)

        # per-partition bias constant ln(8) for the fused exp(bbox)*8
        ln8 = const_pool.tile([128, 1], f32)
        nc.vector.memset(ln8[:], math.log(8.0))

        loop_cm = tc.For_i(0, loop_n, 1) if loop_n else nullcontext()
        with loop_cm:
          for _rep in range(repeat):
            # ---- loads (cls first: it gates the critical path) ----
            bb_t, ct_t, cls_t = {}, {}, {}
            for b, w0, cw in chunks:
                t = cls_pool.tile([128, 64 * C], f32, tag="cls", name="clst")
                cls_t[(b, w0)] = t
                src = cls_d.ap()[b][:, w0 : w0 + cw, :]
                nc.sync.dma_start(
                    t[:, : cw * C], src.rearrange("h w c -> h (w c)")
                )
            for b in range(NB):
                bb_t[b] = io_pool.tile([128, W * 4], f32, tag="bb", name="bbt")
                nc.sync.dma_start(
                    bb_t[b][:], bbox_d.ap()[b].rearrange("h w c -> h (w c)")
                )
            for b in range(NB):
                t = io_pool.tile([128, W], f32, tag="ct", name="ctt")
                ct_t[b] = t
                nc.sync.dma_start(t[:], ctr_d.ap()[b].rearrange("h w c -> h (w c)"))

            # ---- ACT phase 1: exp (one table) ----
            e8 = {}
            for b in range(NB):
                e8[b] = scr_pool.tile([128, W * 4], f32, tag="e8", name="e8t")
                nc.scalar.activation(e8[b][:], bb_t[b][:], Act.Exp, bias=ln8[:, 0:1])

            # ---- DVE: ltrb -> xywh ----
            for b in range(NB):
                e3 = e8[b][:].rearrange("p (w k) -> p w k", k=4)
                lt = e3[:, :, 0:2]
                rb = e3[:, :, 2:4]
                xywh_t = io_pool.tile([128, W * 4], f32, tag="xywh", name="xywht")
                x3 = xywh_t[:].rearrange("p (w k) -> p w k", k=4)
                nc.vector.tensor_tensor(x3[:, :, 2:4], lt, rb, op=Alu.add)
                d_t = scr_pool.tile([128, W * 2], f32, tag="d", name="dt")
                d3 = d_t[:].rearrange("p (w k) -> p w k", k=2)
                nc.vector.tensor_tensor(d3, lt, rb, op=Alu.subtract)
                nc.vector.scalar_tensor_tensor(
                    x3[:, :, 0:2], d3, -0.5, cxy_f3, op0=Alu.mult, op1=Alu.add
                )
                nc.sync.dma_start(
                    xywh_d.ap()[b].rearrange("(h w) k -> h (w k)", h=H), xywh_t[:]
                )

            # ---- cls pipeline: per-chunk reduce/mask; per-batch enc/tree ----
            m_t, mask_b, enc_b = {}, {}, {}
            for b in range(NB):
                m_t[b] = io_pool.tile([128, W], f32, tag="m", name="mt")
                if not fused_enc:
                    mask_b[b] = mask_pool.tile(
                        [128, W * C], bf16, tag="mask", name="maskb"
                    )
                enc_b[b] = enc_pool.tile([128, W * C], bf16, tag="enc", name="encb")
            for b, w0, cw in chunks:
                cls3 = (
                    cls_t[(b, w0)][:, : cw * C]
                    .rearrange("p (w c) -> p w c", c=C)
                )
                m_c = m_t[b][:, w0 : w0 + cw]
                nc.vector.tensor_reduce(
                    m_c, cls3, axis=mybir.AxisListType.X, op=Alu.max
                )
                m_bcast = m_c.unsqueeze(2).broadcast_to([128, cw, C])
                if fused_enc:
                    enc3c = (
                        enc_b[b][:, w0 * C : (w0 + cw) * C]
                        .rearrange("p (w c) -> p w c", c=C)
                    )
                    nc.vector._custom_dve(
                        enc_op, out=enc3c, in0=cls3, in1=m_bcast, s0=float(C)
                    )
                else:
                    mask3 = (
                        mask_b[b][:, w0 * C : (w0 + cw) * C]
                        .rearrange("p (w c) -> p w c", c=C)
                    )
                    nc.vector.tensor_tensor(mask3, cls3, m_bcast, op=Alu.is_ge)

            # per-batch enc + bf16 pairwise max tree + idx
            for b in range(NB):
                encf = enc_b[b][:].rearrange("p (w c) -> p w c", c=C)
                if not fused_enc:
                    maskf = mask_b[b][:].rearrange("p (w c) -> p w c", c=C)
                    iota_bcast = iota_b[:].unsqueeze(1).broadcast_to([128, W, C])
                    nc.vector.tensor_tensor(encf, maskf, iota_bcast, op=Alu.mult)

                tree_t = scr_pool.tile([128, W * 75], bf16, tag="tree", name="treet")
                offs = {40: 0, 20: W * 40, 10: W * 60, 5: W * 70}

                def lvl(n):
                    return tree_t[:, offs[n] : offs[n] + W * n].rearrange(
                        "p (w c) -> p w c", c=n
                    )

                a40, a20, a10, a5 = (lvl(n) for n in (40, 20, 10, 5))
                nc.vector.tensor_tensor(
                    a40, encf[:, :, 0:40], encf[:, :, 40:80], op=Alu.max
                )
                nc.vector.tensor_tensor(
                    a20, a40[:, :, 0:20], a40[:, :, 20:40], op=Alu.max
                )
                nc.vector.tensor_tensor(
                    a10, a20[:, :, 0:10], a20[:, :, 10:20], op=Alu.max
                )
                nc.vector.tensor_tensor(a5, a10[:, :, 0:5], a10[:, :, 5:10], op=Alu.max)
                encm_c = scr_pool.tile([128, W], f32, tag="encm", name="encmc")
                nc.vector.tensor_reduce(
                    encm_c[:], a5, axis=mybir.AxisListType.X, op=Alu.max
                )

                idx_i = io_pool.tile([128, W], i32, tag="idxi", name="idxi")
                nc.scalar.activation(
                    idx_i[:], encm_c[:], Act.Copy,
                    bias=0.0 if fused_enc else float(C), scale=-1.0,
                )
                nc.sync.dma_start(
                    idx_d.ap()[b].rearrange("(h w) -> h w", h=H), idx_i[:]
                )

            # ---- ACT phase 2: sigmoids (one table) ----
            sigc_t, sigm_t = {}, {}
            for b in range(NB):
                sigc = scr_pool.tile([128, W], f32, tag="sigc", name="sigct")
                sigc_t[b] = sigc
                nc.scalar.activation(sigc[:], ct_t[b][:], Act.Sigmoid)
                sigm = scr_pool.tile([128, W], f32, tag="sigm", name="sigmt")
                sigm_t[b] = sigm
                nc.scalar.activation(sigm[:], m_t[b][:], Act.Sigmoid)

            # ---- DVE: sigc * sigm ----
            prod_t = {}
            for b in range(NB):
                prod = scr_pool.tile([128, W], f32, tag="prod", name="prodt")
                prod_t[b] = prod
                nc.vector.tensor_tensor(
                    prod[:], sigc_t[b][:], sigm_t[b][:], op=Alu.mult
                )

            # ---- ACT phase 3: sqrt (one table) + store ----
            for b in range(NB):
                conf_t = io_pool.tile([128, W], f32, tag="conf", name="conft")
                nc.scalar.activation(conf_t[:], prod_t[b][:], Act.Sqrt)
                nc.sync.dma_start(
                    conf_d.ap()[b].rearrange("(h w) -> h w", h=H), conf_t[:]
                )

    nc.compile()
    return nc


def _get_nc():
    if "nc" not in _CACHE:
        _CACHE["nc"] = _build()
    return _CACHE["nc"]


def _run(in_maps, **kwargs):
    from concourse.bass_utils import run_bass_kernel_spmd

    return run_bass_kernel_spmd(_get_nc(), in_maps, list(range(N_CORES)), **kwargs)


def _make_runner_for(nc):
    """Persistent jitted shard_map callable (mirrors bass2jax.run_bass_via_pjrt
    but caches the jit so repeat calls don't retrace/recompile)."""
    import jax
    import concourse.mybir as mybir
    from concourse.bass2jax import (
        _bass_exec_p,
        install_neuronx_cc_hook,
        partition_id_tensor,
    )

    try:
        from jax.experimental.shard_map import shard_map
    except ImportError:
        from jax import shard_map
    from jax.sharding import Mesh, PartitionSpec

    install_neuronx_cc_hook()
    assert nc.dbg_addr is None

    partition_name = (
        nc.partition_id_tensor.name if nc.partition_id_tensor is not None else None
    )
    in_names, out_names, out_avals, zero_outs = [], [], [], []
    for alloc in nc.m.functions[0].allocations:
        if not isinstance(alloc, mybir.MemoryLocationSet):
            continue
        name = alloc.memorylocations[0].name
        if alloc.kind == "ExternalInput":
            if name != partition_name:
                in_names.append(name)
        elif alloc.kind == "ExternalOutput":
            shape = tuple(alloc.tensor_shape)
            dtype = mybir.dt.np(alloc.dtype)
            out_names.append(name)
            out_avals.append(jax.core.ShapedArray(shape, dtype))
            zero_outs.append(np.zeros(shape, dtype))
    n_params = len(in_names)
    all_in_names = in_names + out_names
    if partition_name is not None:
        all_in_names.append(partition_name)

    def _body(*args):
        operands = list(args)
        if partition_name is not None:
            operands.append(partition_id_tensor())
        outs = _bass_exec_p.bind(
            *operands,
            out_avals=tuple(out_avals),
            in_names=tuple(all_in_names),
            out_names=tuple(out_names),
            lowering_input_output_aliases=(),
            sim_require_finite=True,
            sim_require_nnan=True,
            nc=nc,
        )
        return tuple(outs)

    devices = jax.devices()[:N_CORES]
    mesh = Mesh(np.asarray(devices), ("core",))
    n_outs = len(out_names)

    sharded = jax.jit(
        shard_map(
            _body,
            mesh=mesh,
            in_specs=(PartitionSpec("core"),) * (n_params + n_outs),
            out_specs=(PartitionSpec("core"),) * n_outs,
            check_rep=False,
        ),
        donate_argnums=tuple(range(n_params, n_params + n_outs)),
        keep_unused=True,
    )

    def run(bbox, center, cls_logits):
        by_name = {"bbox": bbox, "center": center, "cls": cls_logits}
        ins = [by_name[n] for n in in_names]
        concat_zeros = [
            np.zeros((N_CORES * z.shape[0], *z.shape[1:]), z.dtype) for z in zero_outs
        ]
        return sharded(*ins, *concat_zeros)

    return run, out_names


def _get_runner():
    if "runner" not in _CACHE:
        _CACHE["runner"] = _make_runner_for(_get_nc())
    return _CACHE["runner"]


def kernel(bbox, center, cls_logits):
    bbox = np.ascontiguousarray(np.asarray(bbox, dtype=np.float32))
    center = np.ascontiguousarray(np.asarray(center, dtype=np.float32))
    cls_logits = np.ascontiguousarray(np.asarray(cls_logits, dtype=np.float32))

    run, out_names = _get_runner()
    out_arrs = run(bbox, center, cls_logits)
    by_name = {n: np.asarray(a) for n, a in zip(out_names, out_arrs)}
    xywh = by_name["xywh"]
    idx = by_name["idx"].astype(np.int32)
    conf = by_name["conf"]
    return xywh, idx, conf


# revision 30
# speedup vs baseline: 1.2535x; 1.2535x over previous
"""FCOS detection decode kernel for Trainium2 (8 NeuronCores, data parallel).

Full inputs:
  bbox       [16, 128, 128, 4]  f32
  center     [16, 128, 128, 1]  f32
  cls_logits [16, 128, 128, 80] f32
Full outputs (tuple, matching the reference):
  xywh [16, 16384, 4] f32
  idx  [16, 16384]    int32
  conf [16, 16384]    f32

Sharding: batch dim across 8 cores (2 batches per core), no communication.

Per-core layout: partition dim = H (128 rows), free dim = W*channels.
  - exp/sigmoid/sqrt on ScalarE (ACT)
  - ltrb->xywh linear combinations on VectorE with a precomputed (cx, cy) tile
  - class max via tensor_reduce(max) over C, W-chunked (fine chunks first so
    compute starts as soon as the first cls DMA slice lands)
  - argmax via a runtime-registered custom DVE op:
    enc = select(cls >= max, -(within-page class position), -FLT_MAX) in one
    1x pass (values -79..0, bf16-exact), then a bf16 pairwise-max tree
    (tensor_tensor max is 2x on bf16 while tensor_reduce is always 1x) and
    idx = -max(enc) on ACT. The negated-position encoding makes ties resolve
    to the first (lowest) class index, exactly matching jnp.argmax.
"""

import math

import numpy as np

NB_FULL = 16
N_CORES = 8
NB = NB_FULL // N_CORES  # batches per core
H = 128
W = 128
C = 80

_CACHE = {}



def _register_enc_op():
    """Custom DVE op: enc = select(cls >= m, -within_page_pos, -FLT_MAX).

    One 1x pass replaces the is_ge mask pass + the iota multiply. SubIdx is
    the page (cell) counter, Idx the global stream position, so
    C0*SubIdx - Idx = -(position within the 80-class page). Values are
    -79..0 (bf16-exact); reduce_max(enc) = -argmax with first-occurrence
    ties, matching jnp.argmax.
    """
    import numpy as np
    from concourse import dve_ops
    from concourse.dve_spec import (
        Spec, Src0, Src1, C0, MaxNeg, select, lower, Idx, SubIdx,
    )
    from concourse.dve_uop import DveOpSpec

    for o in dve_ops.OPS:
        if o.name == "ENC_ARGMAX_ANT":
            return o

    def ref(in0, in1, s0, s1, imm2):
        P, S, N = in0.shape
        pos = np.arange(S * N).reshape(1, S, N)
        sub = np.repeat(np.arange(S), N).reshape(1, S, N)
        s0v = float(s0) if not isinstance(s0, np.ndarray) else float(s0.flat[0])
        return np.where(in0 >= in1, s0v * sub - pos, -3.4028235e38)

    spec = Spec(
        body=select(Src0 >= Src1, (C0 * SubIdx) - Idx, MaxNeg), reference=ref
    )
    shas = {}
    for ver in ("v3", "v4"):
        tmp = DveOpSpec(
            name="ENC_ARGMAX_ANT", opcode=0, uops=lower(spec, ver=ver), rd1_en=True
        )
        shas[ver] = tmp.sha(ver)
    op = dve_ops.DveOp("ENC_ARGMAX_ANT", spec, subdim=True, uops_sha=shas)
    dve_ops.OPS.append(op)
    dve_ops._SUB_OPCODE_FOR_NAME[op.name] = (
        dve_ops._CUSTOM_DVE_ROW_BASE + len(dve_ops.OPS) - 1
    )
    dve_ops.CUSTOM_DVE_SPECS[op.name] = op.spec
    return op


def _build(repeat=1, chunk_plan=None, loop_n=0, cls_bufs=4, mask_bufs=1, enc_bufs=2, fused_enc=True):

    """Build the per-core Bass/Tile program.

    chunk_plan: per-batch list of W-chunk widths for the cls pipeline.
      Fine first chunks let compute start early; coarse later chunks cut
      per-instruction overhead.
    repeat / loop_n: benchmarking only (python-unrolled / hardware For_i).
    """
    from contextlib import ExitStack, nullcontext

    import concourse.bacc as bacc
    import concourse.mybir as mybir
    import concourse.tile as tile

    f32 = mybir.dt.float32
    bf16 = mybir.dt.bfloat16
    i32 = mybir.dt.int32
    Alu = mybir.AluOpType
    Act = mybir.ActivationFunctionType

    if chunk_plan is None:
        chunk_plan = [[8, 8, 16, 32, 32, 32], [64, 64]]
    assert len(chunk_plan) == NB and all(sum(p) == W for p in chunk_plan)

    enc_op = None
    if fused_enc:
        try:
            enc_op = _register_enc_op()
        except Exception:
            fused_enc = False  # fall back to stock mask+enc ops

    nc = bacc.Bacc("TRN2", target_bir_lowering=False, debug=False)

    bbox_d = nc.dram_tensor("bbox", [NB, H, W, 4], f32, kind="ExternalInput")
    ctr_d = nc.dram_tensor("center", [NB, H, W, 1], f32, kind="ExternalInput")
    cls_d = nc.dram_tensor("cls", [NB, H, W, C], f32, kind="ExternalInput")
    xywh_d = nc.dram_tensor("xywh", [NB, H * W, 4], f32, kind="ExternalOutput")
    idx_d = nc.dram_tensor("idx", [NB, H * W], i32, kind="ExternalOutput")
    conf_d = nc.dram_tensor("conf", [NB, H * W], f32, kind="ExternalOutput")

    # (batch, w_start, w_width) in pipeline order
    chunks = []
    for b, plan in enumerate(chunk_plan):
        w0 = 0
        for cw in plan:
            chunks.append((b, w0, cw))
            w0 += cw

    with tile.TileContext(nc) as tc, ExitStack() as ctx:
        const_pool = ctx.enter_context(tc.tile_pool(name="const", bufs=1))
        cls_pool = ctx.enter_context(tc.tile_pool(name="cls", bufs=cls_bufs))
        mask_pool = ctx.enter_context(tc.tile_pool(name="maskp", bufs=mask_bufs))
        enc_pool = ctx.enter_context(tc.tile_pool(name="encp", bufs=enc_bufs))
        io_pool = ctx.enter_context(tc.tile_pool(name="io", bufs=2))
        scr_pool = ctx.enter_context(tc.tile_pool(name="scr", bufs=1))

        # Descending iota 80..1 (value 80-c), as bf16 (exact for ints <= 256).
        iota_i = const_pool.tile([128, C], i32)
        nc.gpsimd.iota(iota_i[:], pattern=[[-1, C]], base=C, channel_multiplier=0)
        iota_b = const_pool.tile([128, C], bf16)
        nc.vector.tensor_copy(iota_b[:], iota_i[:])

        # cxy tile [128, W, 2]: (cx=8w+4 along free, cy=8h+4 along partitions)
        cxy_i = const_pool.tile([128, W * 2], i32)
        cxy_i3 = cxy_i[:].rearrange("p (w k) -> p w k", k=2)
        nc.gpsimd.iota(cxy_i3[:, :, 0], pattern=[[1, W]], base=0, channel_multiplier=0)
        nc.gpsimd.iota(cxy_i3[:, :, 1], pattern=[[0, W]], base=0, channel_multiplier=1)
        cxy_f = const_pool.tile([128, W * 2], f32)
        nc.vector.tensor_scalar(cxy_f[:], cxy_i[:], 8.0, 4.0, op0=Alu.mult, op1=Alu.add)
        cxy_f3 = cxy_f[:].rearrange("p (w k) -> p w k", k=2)

        # per-partition bias constant ln(8) for the fused exp(bbox)*8
        ln8 = const_pool.tile([128, 1], f32)
        nc.vector.memset(ln8[:], math.log(8.0))

        loop_cm = tc.For_i(0, loop_n, 1) if loop_n else nullcontext()
        with loop_cm:
          for _rep in range(repeat):
            # ---- loads (cls first: it gates the critical path) ----
            bb_t, ct_t, cls_t = {}, {}, {}
            for b, w0, cw in chunks:
                t = cls_pool.tile([128, 64 * C], f32, tag="cls", name="clst")
                cls_t[(b, w0)] = t
                src = cls_d.ap()[b][:, w0 : w0 + cw, :]
                nc.sync.dma_start(
                    t[:, : cw * C], src.rearrange("h w c -> h (w c)")
                )
            for b in range(NB):
                bb_t[b] = io_pool.tile([128, W * 4], f32, tag="bb", name="bbt")
                nc.sync.dma_start(
                    bb_t[b][:], bbox_d.ap()[b].rearrange("h w c -> h (w c)")
                )
            for b in range(NB):
                t = io_pool.tile([128, W], f32, tag="ct", name="ctt")
                ct_t[b] = t
                nc.sync.dma_start(t[:], ctr_d.ap()[b].rearrange("h w c -> h (w c)"))

            # ---- ACT phase 1: exp (one table) ----
            e8 = {}
            for b in range(NB):
                e8[b] = scr_pool.tile([128, W * 4], f32, tag="e8", name="e8t")
                nc.scalar.activation(e8[b][:], bb_t[b][:], Act.Exp, bias=ln8[:, 0:1])

            # ---- DVE: ltrb -> xywh ----
            for b in range(NB):
                e3 = e8[b][:].rearrange("p (w k) -> p w k", k=4)
                lt = e3[:, :, 0:2]
                rb = e3[:, :, 2:4]
                xywh_t = io_pool.tile([128, W * 4], f32, tag="xywh", name="xywht")
                x3 = xywh_t[:].rearrange("p (w k) -> p w k", k=4)
                nc.vector.tensor_tensor(x3[:, :, 2:4], lt, rb, op=Alu.add)
                d_t = scr_pool.tile([128, W * 2], f32, tag="d", name="dt")
                d3 = d_t[:].rearrange("p (w k) -> p w k", k=2)
                nc.vector.tensor_tensor(d3, lt, rb, op=Alu.subtract)
                nc.vector.scalar_tensor_tensor(
                    x3[:, :, 0:2], d3, -0.5, cxy_f3, op0=Alu.mult, op1=Alu.add
                )
                nc.sync.dma_start(
                    xywh_d.ap()[b].rearrange("(h w) k -> h (w k)", h=H), xywh_t[:]
                )

            # ---- cls pipeline: per-chunk reduce/mask; per-batch enc/tree ----
            m_t, mask_b, enc_b = {}, {}, {}
            for b in range(NB):
                m_t[b] = io_pool.tile([128, W], f32, tag="m", name="mt")
                if not fused_enc:
                    mask_b[b] = mask_pool.tile(
                        [128, W * C], bf16, tag="mask", name="maskb"
                    )
                enc_b[b] = enc_pool.tile([128, W * C], bf16, tag="enc", name="encb")
            for b, w0, cw in chunks:
                cls3 = (
                    cls_t[(b, w0)][:, : cw * C]
                    .rearrange("p (w c) -> p w c", c=C)
                )
                m_c = m_t[b][:, w0 : w0 + cw]
                nc.vector.tensor_reduce(
                    m_c, cls3, axis=mybir.AxisListType.X, op=Alu.max
                )
                m_bcast = m_c.unsqueeze(2).broadcast_to([128, cw, C])
                if fused_enc:
                    enc3c = (
                        enc_b[b][:, w0 * C : (w0 + cw) * C]
                        .rearrange("p (w c) -> p w c", c=C)
                    )
                    nc.vector._custom_dve(
                        enc_op, out=enc3c, in0=cls3, in1=m_bcast, s0=float(C)
                    )
                else:
                    mask3 = (
                        mask_b[b][:, w0 * C : (w0 + cw) * C]
                        .rearrange("p (w c) -> p w c", c=C)
                    )
                    nc.vector.tensor_tensor(mask3, cls3, m_bcast, op=Alu.is_ge)

            # per-batch enc + bf16 pairwise max tree + idx
            for b in range(NB):
                encf = enc_b[b][:].rearrange("p (w c) -> p w c", c=C)
                if not fused_enc:
                    maskf = mask_b[b][:].rearrange("p (w c) -> p w c", c=C)
                    iota_bcast = iota_b[:].unsqueeze(1).broadcast_to([128, W, C])
                    nc.vector.tensor_tensor(encf, maskf, iota_bcast, op=Alu.mult)

                tree_t = scr_pool.tile([128, W * 75], bf16, tag="tree", name="treet")
                offs = {40: 0, 20: W * 40, 10: W * 60, 5: W * 70}

                def lvl(n):
                    return tree_t[:, offs[n] : offs[n] + W * n].rearrange(
                        "p (w c) -> p w c", c=n
                    )

                a40, a20, a10, a5 = (lvl(n) for n in (40, 20, 10, 5))
                nc.vector.tensor_tensor(
                    a40, encf[:, :, 0:40], encf[:, :, 40:80], op=Alu.max
                )
                nc.vector.tensor_tensor(
                    a20, a40[:, :, 0:20], a40[:, :, 20:40], op=Alu.max
                )
                nc.vector.tensor_tensor(
                    a10, a20[:, :, 0:10], a20[:, :, 10:20], op=Alu.max
                )
                nc.vector.tensor_tensor(a5, a10[:, :, 0:5], a10[:, :, 5:10], op=Alu.max)
                encm_c = scr_pool.tile([128, W], f32, tag="encm", name="encmc")
                nc.vector.tensor_reduce(
                    encm_c[:], a5, axis=mybir.AxisListType.X, op=Alu.max
                )

                idx_i = io_pool.tile([128, W], i32, tag="idxi", name="idxi")
                nc.scalar.activation(
                    idx_i[:], encm_c[:], Act.Copy,
                    bias=0.0 if fused_enc else float(C), scale=-1.0,
                )
                nc.sync.dma_start(
                    idx_d.ap()[b].rearrange("(h w) -> h w", h=H), idx_i[:]
                )

            # ---- ACT phase 2: sigmoids (one table) ----
            sigc_t, sigm_t = {}, {}
            for b in range(NB):
                sigc = scr_pool.tile([128, W], f32, tag="sigc", name="sigct")
                sigc_t[b] = sigc
                nc.scalar.activation(sigc[:], ct_t[b][:], Act.Sigmoid)
                sigm = scr_pool.tile([128, W], f32, tag="sigm", name="sigmt")
                sigm_t[b] = sigm
                nc.scalar.activation(sigm[:], m_t[b][:], Act.Sigmoid)

            # ---- DVE: sigc * sigm ----
            prod_t = {}
            for b in range(NB):
                prod = scr_pool.tile([128, W], f32, tag="prod", name="prodt")
                prod_t[b] = prod
                nc.vector.tensor_tensor(
                    prod[:], sigc_t[b][:], sigm_t[b][:], op=Alu.mult
                )

            # ---- ACT phase 3: sqrt (one table) + store ----
            for b in range(NB):
                conf_t = io_pool.tile([128, W], f32, tag="conf", name="conft")
                nc.scalar.activation(conf_t[:], prod_t[b][:], Act.Sqrt)
                nc.sync.dma_start(
                    conf_d.ap()[b].rearrange("(h w) -> h w", h=H), conf_t[:]
                )

    nc.compile()
    return nc


def _get_nc():
    if "nc" not in _CACHE:
        _CACHE["nc"] = _build()
    return _CACHE["nc"]


def _run(in_maps, **kwargs):
    from concourse.bass_utils import run_bass_kernel_spmd

    return run_bass_kernel_spmd(_get_nc(), in_maps, list(range(N_CORES)), **kwargs)


def _make_runner_for(nc):
    """Persistent jitted shard_map callable (mirrors bass2jax.run_bass_via_pjrt
    but caches the jit so repeat calls don't retrace/recompile)."""
    import jax
    import concourse.mybir as mybir
    from concourse.bass2jax import (
        _bass_exec_p,
        install_neuronx_cc_hook,
        partition_id_tensor,
    )

    try:
        from jax.experimental.shard_map import shard_map
    except ImportError:
        from jax import shard_map
    from jax.sharding import Mesh, PartitionSpec

    install_neuronx_cc_hook()
    assert nc.dbg_addr is None

    partition_name = (
        nc.partition_id_tensor.name if nc.partition_id_tensor is not None else None
    )
    in_names, out_names, out_avals, zero_outs = [], [], [], []
    for alloc in nc.m.functions[0].allocations:
        if not isinstance(alloc, mybir.MemoryLocationSet):
            continue
        name = alloc.memorylocations[0].name
        if alloc.kind == "ExternalInput":
            if name != partition_name:
                in_names.append(name)
        elif alloc.kind == "ExternalOutput":
            shape = tuple(alloc.tensor_shape)
            dtype = mybir.dt.np(alloc.dtype)
            out_names.append(name)
            out_avals.append(jax.core.ShapedArray(shape, dtype))
            zero_outs.append(np.zeros(shape, dtype))
    n_params = len(in_names)
    all_in_names = in_names + out_names
    if partition_name is not None:
        all_in_names.append(partition_name)

    def _body(*args):
        operands = list(args)
        if partition_name is not None:
            operands.append(partition_id_tensor())
        outs = _bass_exec_p.bind(
            *operands,
            out_avals=tuple(out_avals),
            in_names=tuple(all_in_names),
            out_names=tuple(out_names),
            lowering_input_output_aliases=(),
            sim_require_finite=True,
            sim_require_nnan=True,
            nc=nc,
        )
        return tuple(outs)

    devices = jax.devices()[:N_CORES]
    mesh = Mesh(np.asarray(devices), ("core",))
    n_outs = len(out_names)

    sharded = jax.jit(
        shard_map(
            _body,
            mesh=mesh,
            in_specs=(PartitionSpec("core"),) * (n_params + n_outs),
            out_specs=(PartitionSpec("core"),) * n_outs,
            check_rep=False,
        ),
        donate_argnums=tuple(range(n_params, n_params + n_outs)),
        keep_unused=True,
    )

    def run(bbox, center, cls_logits):
        by_name = {"bbox": bbox, "center": center, "cls": cls_logits}
        ins = [by_name[n] for n in in_names]
        concat_zeros = [
            np.zeros((N_CORES * z.shape[0], *z.shape[1:]), z.dtype) for z in zero_outs
        ]
        return sharded(*ins, *concat_zeros)

    return run, out_names


def _get_runner():
    if "runner" not in _CACHE:
        _CACHE["runner"] = _make_runner_for(_get_nc())
    return _CACHE["runner"]


def kernel(bbox, center, cls_logits):
    bbox = np.ascontiguousarray(np.asarray(bbox, dtype=np.float32))
    center = np.ascontiguousarray(np.asarray(center, dtype=np.float32))
    cls_logits = np.ascontiguousarray(np.asarray(cls_logits, dtype=np.float32))

    run, out_names = _get_runner()
    out_arrs = run(bbox, center, cls_logits)
    by_name = {n: np.asarray(a) for n, a in zip(out_names, out_arrs)}
    xywh = by_name["xywh"]
    idx = by_name["idx"].astype(np.int32)
    conf = by_name["conf"]
    return xywh, idx, conf
